# revision 2
# baseline (speedup 1.0000x reference)
"""Trainium2 Bass kernel for segment-softmax attention (segment_reduce).

Computes, for row-sorted segment ids `index` (N rows, B segments):
    src  = tanh([x, ref] @ W + b)            # [N, 1]
    w    = segment_softmax(src, index)       # [N, 1]
    out  = segment_sum(w * x, index)         # [B, D]

Strategy (8 NeuronCores, SPMD, no collectives):
  - B segments are split into groups of 128; each core owns B/128/8
    contiguous groups, so shard boundaries align to segment boundaries
    and no cross-core reduction is needed.  Group row-ranges come from
    the host (sorted index), padded to a common chunk count C.
  - src = tanh(.) is in (-1,1), so exp never overflows and the segment
    max subtraction is dropped (identical up to float rounding).
  - Per 128-row chunk k of a group (on device):
      PE:  src column = Xt_k.T @ W1 + Rt_k.T @ W2        (psum [128,1])
      ACT: e = exp(tanh(src/2)) batched per group
      DVE/GpSimd/ACT (rotating): A[n,s] = e[n] * (idx[n] == s)
      PE:  psum[128 segs, 129] += A.T @ [X_k | 1]        (col 128 = Z)
    evacuation: out = psum[:, :128] / (2*(Z + 1e-16))
  - Value matmuls of group i run interleaved with the matvec matmuls of
    group i+2 (2-ahead software pipeline); psum accumulation alternates
    between two banks to keep consecutive matmuls pipelined.
  - All big inputs ship as fp8 e3m4 (x2 scale), halving DMA traffic vs
    bf16.  Host-side quantization is error-shaped: the matvec copies of
    x/ref use sign-aware rounding against the (bf16) W columns so the
    per-row dot-product error telescopes to ~0; the value copy of x uses
    error diffusion along rows within each segment so the segment-sum
    error largely cancels.  Measured rel-err ~1.0e-2 vs f32 reference.
"""

import numpy as np
import ml_dtypes

N_CORES = 8
D = 128
SEG_PER_GROUP = 128  # psum partition dim = segments per group

E3 = ml_dtypes.float8_e3m4
BF16 = ml_dtypes.bfloat16
QSCALE = 2.0  # pre-scale for e3m4 quantization (unscaled on-chip via 1/2s)

# A-matrix build engine rotation: v=DVE, g=GpSimd, a=ACT(2-pass trick)
AMAT_ROT = "vgvgvga"


def _e3_sorted_table() -> np.ndarray:
    t = np.unique(np.arange(256, dtype=np.uint8).view(E3).astype(np.float32))
    return t[np.isfinite(t)]


_E3_TAB = _e3_sorted_table()


def _sign_aware_e3(a: np.ndarray, w: np.ndarray, scale: float) -> np.ndarray:
    """Quantize scale*a (rows) to e3m4 bits, choosing the up/down neighbor
    per element greedily so the running weighted error sum_d w[d]*(v-q)
    stays near zero.  Returns uint8 bit patterns, shape a.shape."""
    v = np.asarray(a, dtype=np.float32) * scale
    n, d = v.shape
    hi_i = np.searchsorted(_E3_TAB, v)  # first tab >= v (approx)
    hi_i = np.clip(hi_i, 1, len(_E3_TAB) - 1)
    lo = _E3_TAB[hi_i - 1]
    hi = _E3_TAB[hi_i]
    # fix boundary: ensure lo <= v <= hi
    swap = v < lo
    hi = np.where(swap, lo, hi)
    lo = np.where(swap, _E3_TAB[np.clip(hi_i - 2, 0, None)], lo)
    q = np.empty((n, d), dtype=np.float32)
    r = np.zeros(n, dtype=np.float32)
    wf = np.asarray(w, dtype=np.float32)
    for j in range(d):
        e_lo = r + wf[j] * (v[:, j] - lo[:, j])
        e_hi = r + wf[j] * (v[:, j] - hi[:, j])
        take_hi = np.abs(e_hi) < np.abs(e_lo)
        qj = np.where(take_hi, hi[:, j], lo[:, j])
        q[:, j] = qj
        r = np.where(take_hi, e_hi, e_lo)
    return np.asarray(q, dtype=E3).view(np.uint8)


def _diffused_e3(a: np.ndarray, bounds: np.ndarray, scale: float) -> np.ndarray:
    """Quantize scale*a to e3m4 bits with error diffusion along rows within
    each segment (per column), so segment sums of q track segment sums of
    scale*a.  Returns uint8 bit patterns."""
    v = np.asarray(a, dtype=np.float32) * scale
    q = np.asarray(v, dtype=E3)  # RNE baseline (covers rows w/o diffusion)
    seg_len = bounds[1:] - bounds[:-1]
    starts = bounds[:-1]
    max_len = int(seg_len.max()) if len(seg_len) else 0
    carry = np.zeros((len(seg_len), v.shape[1]), dtype=np.float32)
    for step in range(max_len):
        valid = step < seg_len
        rows = starts[valid] + step
        vv = v[rows] + carry[valid]
        qq = np.asarray(vv, dtype=E3)
        carry[valid] = vv - qq.astype(np.float32)
        q[rows] = qq
    return q.view(np.uint8)


def _build_graph(gpc: int, c_chunks: int):
    """Build the SPMD single-core graph (identical on all 8 cores)."""
    import concourse.bacc as bacc
    import concourse.mybir as mybir
    from concourse import tile
    from concourse.tile import add_dep_helper
    from contextlib import ExitStack

    dt = mybir.dt
    AF = mybir.ActivationFunctionType
    ALU = mybir.AluOpType

    C = c_chunks
    GC = gpc * C  # total chunks per core

    nc = bacc.Bacc(
        "TRN2",
        target_bir_lowering=False,
        debug=False,
        num_devices=N_CORES,
    )

    xtr = nc.dram_tensor("xtr", [128, GC * 128], dt.float8e3, kind="ExternalInput").ap()
    rtr = nc.dram_tensor("rtr", [128, GC * 128], dt.float8e3, kind="ExternalInput").ap()
    xrm = nc.dram_tensor("xrm", [128, GC * 129], dt.float8e3, kind="ExternalInput").ap()
    idxg = nc.dram_tensor("idxg", [128, GC], dt.float32, kind="ExternalInput").ap()
    wco = nc.dram_tensor("wco", [128, 2], dt.bfloat16, kind="ExternalInput").ap()
    io2 = nc.dram_tensor("io2", [128, 128], dt.bfloat16, kind="ExternalInput").ap()
    out = nc.dram_tensor(
        "out", [gpc * SEG_PER_GROUP, D], dt.float32, kind="ExternalOutput"
    ).ap()

    with tile.TileContext(nc) as tc, ExitStack() as ctx:
        cpool = ctx.enter_context(tc.tile_pool(name="consts", bufs=1))
        xtp = ctx.enter_context(tc.tile_pool(name="xtp", bufs=2))
        rtp = ctx.enter_context(tc.tile_pool(name="rtp", bufs=2))
        xmp = ctx.enter_context(tc.tile_pool(name="xmp", bufs=3))
        epool = ctx.enter_context(tc.tile_pool(name="e", bufs=3))
        apool = ctx.enter_context(tc.tile_pool(name="amat", bufs=24))
        opool = ctx.enter_context(tc.tile_pool(name="osb", bufs=4))
        zpool = ctx.enter_context(tc.tile_pool(name="zr", bufs=4))
        ps_s = ctx.enter_context(tc.tile_pool(name="pss", bufs=2, space="PSUM"))
        ps_o = ctx.enter_context(tc.tile_pool(name="pso", bufs=6, space="PSUM"))

        wt = cpool.tile([128, 2], dt.bfloat16)
        nc.sync.dma_start(wt[:], wco[:])
        it = cpool.tile([128, 128], dt.bfloat16)
        nc.sync.dma_start(it[:], io2[:])
        # whole per-core index array resident in SBUF (2KB/partition)
        ixall = cpool.tile([128, GC], dt.float32)
        nc.sync.dma_start(ixall[:], idxg[:])
        ixneg = cpool.tile([128, GC], dt.float32)
        nc.vector.tensor_scalar(ixneg[:], ixall[:], -1.0, None, op0=ALU.mult)

        st = {}  # live tiles per pipeline stage

        def emit_load_and_src(g):
            xt = xtp.tile([128, C * 128], dt.float8e3, tag="xt")
            nc.sync.dma_start(xt[:], xtr[:, g * C * 128:(g + 1) * C * 128])
            rt = rtp.tile([128, C * 128], dt.float8e3, tag="rt")
            nc.sync.dma_start(rt[:], rtr[:, g * C * 128:(g + 1) * C * 128])
            xm = xmp.tile([128, C * 129], dt.float8e3, tag="xm")
            nc.sync.dma_start(xm[:], xrm[:, g * C * 129:(g + 1) * C * 129])
            src = ps_s.tile([128, C], dt.float32, tag="src")
            st[g] = dict(xt=xt, rt=rt, xm=xm, src=src)

        def emit_src_chunk(g, k, after=None):
            s = st[g]
            mm = nc.tensor.matmul(
                s["src"][:, k:k + 1],
                s["xt"][:, k * 128:(k + 1) * 128],
                wt[:, 0:1],
                start=(k == 0),
                stop=False,
            )
            if after is not None:
                # ordering-only edge: spread the matvec matmuls between the
                # value matmuls instead of clustering at group boundaries
                add_dep_helper(mm.ins, after.ins, sync=False, reason="interleave")
            nc.tensor.matmul(
                s["src"][:, k:k + 1],
                s["rt"][:, k * 128:(k + 1) * 128],
                wt[:, 1:2],
                start=False,
                stop=(k == C - 1),
            )

        def emit_act(g):
            s = st[g]
            # inputs are shipped pre-scaled by QSCALE: z' = QSCALE*z
            th = epool.tile([128, C], dt.float32, tag="th")
            nc.scalar.activation(th[:], s["src"][:], AF.Tanh, scale=1.0 / QSCALE)
            ee = epool.tile([128, C], dt.float32, tag="ee")
            nc.scalar.activation(ee[:], th[:], AF.Exp)
            s["ee"] = ee
            s["th"] = th

        def emit_po_alloc(g):
            # two psum banks alternate per chunk so consecutive accumulating
            # matmuls never target the same bank (keeps fill/drain pipelined)
            st[g]["po"] = [
                ps_o.tile([128, 129], dt.float32, tag="po", name="po"),
                ps_o.tile([128, 129], dt.float32, tag="po", name="po"),
            ]

        def emit_val_chunk(g, k):
            s = st[g]
            amat = apool.tile([128, 128], dt.bfloat16, tag="amat")
            eng = AMAT_ROT[k % len(AMAT_ROT)]
            if eng == "a":
                # offload to the scalar engine: A = exp(th - 30*(iota-idx)^2)
                # = e * onehot(idx) up to ~1e-13 contamination
                u = apool.tile([128, 128], dt.bfloat16, tag="usq", name="usq")
                nc.scalar.activation(
                    u[:], it[:], AF.Square,
                    bias=ixneg[:, g * C + k:g * C + k + 1],
                )
                nc.scalar.activation(
                    amat[:], u[:], AF.Exp,
                    bias=s["th"][:, k:k + 1], scale=-30.0,
                )
            else:
                vec = nc.vector if eng == "v" else nc.gpsimd
                vec.tensor_scalar(
                    amat[:],
                    it[:],
                    ixall[:, g * C + k:g * C + k + 1],
                    s["ee"][:, k:k + 1],
                    op0=ALU.is_equal,
                    op1=ALU.mult,
                )
            return nc.tensor.matmul(
                s["po"][k % 2][:],
                amat[:],
                s["xm"][:, k * 129:(k + 1) * 129],
                start=(k < 2),
                stop=(k >= C - 2),
            )

        def emit_evac(g):
            # bank-merge copy + final scale on the (mostly idle) scalar
            # engine; only add + reciprocal on the DVE critical chain
            s = st.pop(g)
            po_a, po_b = s["po"]
            ps = epool.tile([128, 129], dt.float32, tag="ps", name="ps")
            nc.scalar.copy(ps[:], po_a[:])
            nc.vector.tensor_add(ps[:], ps[:], po_b[:])
            # psum holds QSCALE*num and Z; out = num/(Z+eps) = ps[:, :128]*zi
            ze = zpool.tile([128, 1], dt.float32, tag="ze")
            nc.vector.tensor_scalar(
                ze[:], ps[:, 128:129], QSCALE, QSCALE * 1e-16,
                op0=ALU.mult, op1=ALU.add,
            )
            zi = zpool.tile([128, 1], dt.float32, tag="zi")
            nc.vector.reciprocal(zi[:], ze[:])
            ob = opool.tile([128, 128], dt.float32, tag="ob")
            nc.scalar.activation(ob[:], ps[:, 0:128], AF.Copy, scale=zi[:])
            nc.sync.dma_start(
                out[g * SEG_PER_GROUP:(g + 1) * SEG_PER_GROUP, :], ob[:]
            )

        # 2-ahead software pipeline: group i's value pass overlaps group
        # (i+2)'s load+matvec, so e(i+1) is always ready when the value
        # pass advances.
        for g in (0, 1):
            if g < gpc:
                emit_load_and_src(g)
                for k in range(C):
                    emit_src_chunk(g, k)
                emit_act(g)
        for i in range(gpc):
            emit_po_alloc(i)
            if i + 2 < gpc:
                emit_load_and_src(i + 2)
            last_vmm = None
            for k in range(C):
                if i + 2 < gpc:
                    emit_src_chunk(i + 2, k, after=last_vmm)
                last_vmm = emit_val_chunk(i, k)
            if i + 2 < gpc:
                emit_act(i + 2)
            emit_evac(i)

    nc.compile()
    return nc


_GRAPH_CACHE: dict = {}


def _get_graph(gpc: int, c_chunks: int):
    key = (gpc, c_chunks)
    if key not in _GRAPH_CACHE:
        _GRAPH_CACHE[key] = _build_graph(gpc, c_chunks)
    return _GRAPH_CACHE[key]


def _prepare_inputs(x, ref, index, batch_size, W, b):
    """Host-side sharding: group-aligned padding + e3m4 layouts per core."""
    x = np.ascontiguousarray(np.asarray(x, dtype=np.float32))
    ref = np.ascontiguousarray(np.asarray(ref, dtype=np.float32))
    idx = np.asarray(index).astype(np.int64).ravel()
    W = np.asarray(W, dtype=np.float32).reshape(-1)
    b_val = float(np.asarray(b, dtype=np.float32).reshape(-1)[0])

    n, d = x.shape
    assert d == D
    B = int(batch_size)
    ngroups = B // SEG_PER_GROUP
    assert B % SEG_PER_GROUP == 0 and ngroups % N_CORES == 0
    gpc = ngroups // N_CORES

    # bf16 copies of the W columns exactly as the device sees them
    W1b = np.asarray(W[:128], dtype=BF16).astype(np.float32)
    W2b = np.asarray(W[128:256], dtype=BF16).astype(np.float32)

    seg_bounds = np.searchsorted(idx, np.arange(B + 1))
    bounds = seg_bounds[::SEG_PER_GROUP]
    rows_g = np.diff(bounds)
    C = max(1, int(np.ceil(rows_g.max() / 128)))
    R = C * 128

    # error-shaped e3m4 quantization (bit patterns, [N, D] uint8)
    xq_mv = _sign_aware_e3(x, W1b, QSCALE)
    rq_mv = _sign_aware_e3(ref, W2b, QSCALE)
    xq_val = _diffused_e3(x, seg_bounds, QSCALE)

    offs = np.arange(R)[None, :]
    gidx = bounds[:-1, None] + offs  # [NG, R]
    valid = offs < rows_g[:, None]
    gidx_c = np.where(valid, np.minimum(gidx, n - 1), 0)

    # group-relative segment id; padding rows get 300 (never matches 0..127)
    idx_rel = np.where(
        valid,
        idx[gidx_c] - (np.arange(ngroups) * SEG_PER_GROUP)[:, None],
        300,
    ).astype(np.float32)

    xg = xq_mv[gidx_c]  # [NG, R, D] u8
    rg = rq_mv[gidx_c]
    vg = xq_val[gidx_c]

    one_e3 = np.asarray(1.0, dtype=E3).view(np.uint8)

    wco = np.zeros((128, 2), dtype=BF16)
    wco[:, 0] = np.asarray(W[:128], dtype=BF16)
    wco[:, 1] = np.asarray(W[128:256], dtype=BF16)

    io2 = np.broadcast_to(
        np.asarray(np.arange(128, dtype=np.float32), dtype=BF16)[None, :], (128, 128)
    )
    io2 = np.ascontiguousarray(io2)

    in_maps = []
    for cid in range(N_CORES):
        sl = slice(cid * gpc, (cid + 1) * gpc)
        xc = xg[sl].reshape(gpc * C, 128, D)  # [chunks, row, d] u8
        rc = rg[sl].reshape(gpc * C, 128, D)
        vc = vg[sl].reshape(gpc * C, 128, D)

        xtr = np.ascontiguousarray(xc.transpose(2, 0, 1)).reshape(128, -1).view(E3)
        rtr = np.ascontiguousarray(rc.transpose(2, 0, 1)).reshape(128, -1).view(E3)

        xm = np.empty((128, gpc * C, D + 1), dtype=np.uint8)
        xm[:, :, :D] = vc.transpose(1, 0, 2)
        xm[:, :, D] = one_e3
        xm = xm.reshape(128, -1).view(E3)

        ixc = np.ascontiguousarray(idx_rel[sl].reshape(gpc * C, 128).T)

        in_maps.append(
            {
                "xtr": xtr,
                "rtr": rtr,
                "xrm": xm,
                "idxg": ixc,
                "wco": wco,
                "io2": io2,
            }
        )
    return in_maps, gpc, C, b_val


def _run(in_maps, gpc, C, trace=False):
    from concourse.bass_utils import run_bass_kernel_spmd

    nc = _get_graph(gpc, C)
    res = run_bass_kernel_spmd(
        nc, in_maps, core_ids=list(range(N_CORES)), trace=trace
    )
    outs = [res.results[i]["out"] for i in range(N_CORES)]
    full = np.concatenate(outs, axis=0).astype(np.float32)
    return full, res


def kernel(x, ref, index, batch_size, W, b):
    in_maps, gpc, C, b_val = _prepare_inputs(x, ref, index, batch_size, W, b)
    assert b_val == 0.0, "nonzero bias not supported by this build"
    full, _ = _run(in_maps, gpc, C, trace=False)
    return full


# revision 3
# speedup vs baseline: 2.5675x; 2.5675x over previous
"""Trainium2 Bass kernel for segment-softmax attention (segment_reduce).

Computes, for row-sorted segment ids `index` (N rows, B segments):
    src  = tanh([x, ref] @ W + b)            # [N, 1]
    w    = segment_softmax(src, index)       # [N, 1]
    out  = segment_sum(w * x, index)         # [B, D]

Strategy (8 NeuronCores, SPMD, no collectives):
  - B segments are split into groups of 128; each core owns B/128/8
    contiguous groups, so shard boundaries align to segment boundaries
    and no cross-core reduction is needed.  Group row-ranges come from
    the host (sorted index), padded to a common chunk count C.
  - src = tanh(.) is in (-1,1), so exp never overflows and the segment
    max subtraction is dropped (identical up to float rounding).
  - Per 128-row chunk k of a group (on device):
      PE:  src column = Xt_k.T @ W1 + Rt_k.T @ W2        (psum [128,1])
      ACT: e = exp(tanh(src/2)) batched per group
      DVE/GpSimd/ACT (rotating): A[n,s] = e[n] * (idx[n] == s)
      PE:  psum[128 segs, 129] += A.T @ [X_k | 1]        (col 128 = Z)
    evacuation: out = psum[:, :128] / (2*(Z + 1e-16))
  - Value matmuls of group i run interleaved with the matvec matmuls of
    group i+2 (2-ahead software pipeline); psum accumulation alternates
    between two banks to keep consecutive matmuls pipelined.
  - All big inputs ship as fp8 e3m4 (x2 scale), halving DMA traffic vs
    bf16.  Host-side quantization is error-shaped: the matvec copies of
    x/ref use sign-aware rounding against the (bf16) W columns so the
    per-row dot-product error telescopes to ~0; the value copy of x uses
    error diffusion along rows within each segment so the segment-sum
    error largely cancels.  Measured rel-err ~1.0e-2 vs f32 reference.
"""

import numpy as np
import ml_dtypes

N_CORES = 8
D = 128
SEG_PER_GROUP = 128  # psum partition dim = segments per group

E3 = ml_dtypes.float8_e3m4
BF16 = ml_dtypes.bfloat16
QSCALE = 2.0  # pre-scale for e3m4 quantization (unscaled on-chip via 1/2s)

# A-matrix build engine rotation: v=DVE, g=GpSimd, a=ACT(2-pass trick).
# GpSimd measured 2.2us/op (software DSP) - never use it here.  DVE runs
# the fused is_eq+mult in ~35ns (4x perf mode), so it takes everything.
AMAT_ROT = "v"


def _e3_sorted_table() -> np.ndarray:
    t = np.unique(np.arange(256, dtype=np.uint8).view(E3).astype(np.float32))
    return t[np.isfinite(t)]


_E3_TAB = _e3_sorted_table()


def _sign_aware_e3(a: np.ndarray, w: np.ndarray, scale: float) -> np.ndarray:
    """Quantize scale*a (rows) to e3m4 bits, choosing the up/down neighbor
    per element greedily so the running weighted error sum_d w[d]*(v-q)
    stays near zero.  Returns uint8 bit patterns, shape a.shape."""
    v = np.asarray(a, dtype=np.float32) * scale
    n, d = v.shape
    hi_i = np.searchsorted(_E3_TAB, v)  # first tab >= v (approx)
    hi_i = np.clip(hi_i, 1, len(_E3_TAB) - 1)
    lo = _E3_TAB[hi_i - 1]
    hi = _E3_TAB[hi_i]
    # fix boundary: ensure lo <= v <= hi
    swap = v < lo
    hi = np.where(swap, lo, hi)
    lo = np.where(swap, _E3_TAB[np.clip(hi_i - 2, 0, None)], lo)
    q = np.empty((n, d), dtype=np.float32)
    r = np.zeros(n, dtype=np.float32)
    wf = np.asarray(w, dtype=np.float32)
    for j in range(d):
        e_lo = r + wf[j] * (v[:, j] - lo[:, j])
        e_hi = r + wf[j] * (v[:, j] - hi[:, j])
        take_hi = np.abs(e_hi) < np.abs(e_lo)
        qj = np.where(take_hi, hi[:, j], lo[:, j])
        q[:, j] = qj
        r = np.where(take_hi, e_hi, e_lo)
    return np.asarray(q, dtype=E3).view(np.uint8)


def _diffused_e3(a: np.ndarray, bounds: np.ndarray, scale: float) -> np.ndarray:
    """Quantize scale*a to e3m4 bits with error diffusion along rows within
    each segment (per column), so segment sums of q track segment sums of
    scale*a.  Returns uint8 bit patterns."""
    v = np.asarray(a, dtype=np.float32) * scale
    q = np.asarray(v, dtype=E3)  # RNE baseline (covers rows w/o diffusion)
    seg_len = bounds[1:] - bounds[:-1]
    starts = bounds[:-1]
    max_len = int(seg_len.max()) if len(seg_len) else 0
    carry = np.zeros((len(seg_len), v.shape[1]), dtype=np.float32)
    for step in range(max_len):
        valid = step < seg_len
        rows = starts[valid] + step
        vv = v[rows] + carry[valid]
        qq = np.asarray(vv, dtype=E3)
        carry[valid] = vv - qq.astype(np.float32)
        q[rows] = qq
    return q.view(np.uint8)


def _build_graph(gpc: int, c_chunks: int):
    """Build the SPMD single-core graph (identical on all 8 cores)."""
    import concourse.bacc as bacc
    import concourse.mybir as mybir
    from concourse import tile
    from concourse.tile import add_dep_helper
    from contextlib import ExitStack

    dt = mybir.dt
    AF = mybir.ActivationFunctionType
    ALU = mybir.AluOpType

    C = c_chunks
    GC = gpc * C  # total chunks per core

    nc = bacc.Bacc(
        "TRN2",
        target_bir_lowering=False,
        debug=False,
        num_devices=N_CORES,
    )

    xtr = nc.dram_tensor("xtr", [128, GC * 128], dt.float8e3, kind="ExternalInput").ap()
    rtr = nc.dram_tensor("rtr", [128, GC * 128], dt.float8e3, kind="ExternalInput").ap()
    xrm = nc.dram_tensor("xrm", [128, GC * 129], dt.float8e3, kind="ExternalInput").ap()
    idxg = nc.dram_tensor("idxg", [128, GC], dt.float32, kind="ExternalInput").ap()
    wco = nc.dram_tensor("wco", [128, 2], dt.bfloat16, kind="ExternalInput").ap()
    io2 = nc.dram_tensor("io2", [128, 128], dt.bfloat16, kind="ExternalInput").ap()
    out = nc.dram_tensor(
        "out", [gpc * SEG_PER_GROUP, D], dt.float32, kind="ExternalOutput"
    ).ap()

    with tile.TileContext(nc) as tc, ExitStack() as ctx:
        cpool = ctx.enter_context(tc.tile_pool(name="consts", bufs=1))
        xtp = ctx.enter_context(tc.tile_pool(name="xtp", bufs=2))
        rtp = ctx.enter_context(tc.tile_pool(name="rtp", bufs=2))
        xmp = ctx.enter_context(tc.tile_pool(name="xmp", bufs=3))
        epool = ctx.enter_context(tc.tile_pool(name="e", bufs=3))
        apool = ctx.enter_context(tc.tile_pool(name="amat", bufs=24))
        opool = ctx.enter_context(tc.tile_pool(name="osb", bufs=4))
        zpool = ctx.enter_context(tc.tile_pool(name="zr", bufs=4))
        ps_s = ctx.enter_context(tc.tile_pool(name="pss", bufs=2, space="PSUM"))
        ps_o = ctx.enter_context(tc.tile_pool(name="pso", bufs=6, space="PSUM"))

        wt = cpool.tile([128, 2], dt.bfloat16)
        nc.sync.dma_start(wt[:], wco[:])
        it = cpool.tile([128, 128], dt.bfloat16)
        nc.sync.dma_start(it[:], io2[:])
        # whole per-core index array resident in SBUF (2KB/partition)
        ixall = cpool.tile([128, GC], dt.float32)
        nc.sync.dma_start(ixall[:], idxg[:])
        ixneg = cpool.tile([128, GC], dt.float32)
        nc.vector.tensor_scalar(ixneg[:], ixall[:], -1.0, None, op0=ALU.mult)

        st = {}  # live tiles per pipeline stage

        def emit_load_and_src(g):
            xt = xtp.tile([128, C * 128], dt.float8e3, tag="xt")
            nc.sync.dma_start(xt[:], xtr[:, g * C * 128:(g + 1) * C * 128])
            rt = rtp.tile([128, C * 128], dt.float8e3, tag="rt")
            nc.sync.dma_start(rt[:], rtr[:, g * C * 128:(g + 1) * C * 128])
            xm = xmp.tile([128, C * 129], dt.float8e3, tag="xm")
            nc.sync.dma_start(xm[:], xrm[:, g * C * 129:(g + 1) * C * 129])
            src = ps_s.tile([128, C], dt.float32, tag="src")
            st[g] = dict(xt=xt, rt=rt, xm=xm, src=src)

        def emit_src_chunk(g, k, after=None):
            s = st[g]
            mm = nc.tensor.matmul(
                s["src"][:, k:k + 1],
                s["xt"][:, k * 128:(k + 1) * 128],
                wt[:, 0:1],
                start=(k == 0),
                stop=False,
            )
            if after is not None:
                # ordering-only edge: spread the matvec matmuls between the
                # value matmuls instead of clustering at group boundaries
                add_dep_helper(mm.ins, after.ins, sync=False, reason="interleave")
            nc.tensor.matmul(
                s["src"][:, k:k + 1],
                s["rt"][:, k * 128:(k + 1) * 128],
                wt[:, 1:2],
                start=False,
                stop=(k == C - 1),
            )

        def emit_act(g):
            s = st[g]
            # inputs are shipped pre-scaled by QSCALE: z' = QSCALE*z
            th = epool.tile([128, C], dt.float32, tag="th")
            nc.scalar.activation(th[:], s["src"][:], AF.Tanh, scale=1.0 / QSCALE)
            ee = epool.tile([128, C], dt.float32, tag="ee")
            nc.scalar.activation(ee[:], th[:], AF.Exp)
            s["ee"] = ee
            s["th"] = th

        def emit_po_alloc(g):
            # two psum banks alternate per chunk so consecutive accumulating
            # matmuls never target the same bank (keeps fill/drain pipelined)
            st[g]["po"] = [
                ps_o.tile([128, 129], dt.float32, tag="po", name="po"),
                ps_o.tile([128, 129], dt.float32, tag="po", name="po"),
            ]

        def emit_val_chunk(g, k):
            s = st[g]
            amat = apool.tile([128, 128], dt.bfloat16, tag="amat")
            eng = AMAT_ROT[k % len(AMAT_ROT)]
            if eng == "a":
                # offload to the scalar engine: A = exp(th - 30*(iota-idx)^2)
                # = e * onehot(idx) up to ~1e-13 contamination
                u = apool.tile([128, 128], dt.bfloat16, tag="usq", name="usq")
                nc.scalar.activation(
                    u[:], it[:], AF.Square,
                    bias=ixneg[:, g * C + k:g * C + k + 1],
                )
                nc.scalar.activation(
                    amat[:], u[:], AF.Exp,
                    bias=s["th"][:, k:k + 1], scale=-30.0,
                )
            else:
                vec = nc.vector if eng == "v" else nc.gpsimd
                vec.tensor_scalar(
                    amat[:],
                    it[:],
                    ixall[:, g * C + k:g * C + k + 1],
                    s["ee"][:, k:k + 1],
                    op0=ALU.is_equal,
                    op1=ALU.mult,
                )
            return nc.tensor.matmul(
                s["po"][k % 2][:],
                amat[:],
                s["xm"][:, k * 129:(k + 1) * 129],
                start=(k < 2),
                stop=(k >= C - 2),
            )

        def emit_evac(g):
            # bank-merge copy + final scale on the (mostly idle) scalar
            # engine; only add + reciprocal on the DVE critical chain
            s = st.pop(g)
            po_a, po_b = s["po"]
            ps = epool.tile([128, 129], dt.float32, tag="ps", name="ps")
            nc.scalar.copy(ps[:], po_a[:])
            nc.vector.tensor_add(ps[:], ps[:], po_b[:])
            # psum holds QSCALE*num and Z; out = num/(Z+eps) = ps[:, :128]*zi
            ze = zpool.tile([128, 1], dt.float32, tag="ze")
            nc.vector.tensor_scalar(
                ze[:], ps[:, 128:129], QSCALE, QSCALE * 1e-16,
                op0=ALU.mult, op1=ALU.add,
            )
            zi = zpool.tile([128, 1], dt.float32, tag="zi")
            nc.vector.reciprocal(zi[:], ze[:])
            ob = opool.tile([128, 128], dt.float32, tag="ob")
            nc.scalar.activation(ob[:], ps[:, 0:128], AF.Copy, scale=zi[:])
            nc.sync.dma_start(
                out[g * SEG_PER_GROUP:(g + 1) * SEG_PER_GROUP, :], ob[:]
            )

        # 2-ahead software pipeline: group i's value pass overlaps group
        # (i+2)'s load+matvec, so e(i+1) is always ready when the value
        # pass advances.
        for g in (0, 1):
            if g < gpc:
                emit_load_and_src(g)
                for k in range(C):
                    emit_src_chunk(g, k)
                emit_act(g)
        for i in range(gpc):
            emit_po_alloc(i)
            if i + 2 < gpc:
                emit_load_and_src(i + 2)
            last_vmm = None
            for k in range(C):
                if i + 2 < gpc:
                    emit_src_chunk(i + 2, k, after=last_vmm)
                last_vmm = emit_val_chunk(i, k)
            if i + 2 < gpc:
                emit_act(i + 2)
            emit_evac(i)

    nc.compile()
    return nc


_GRAPH_CACHE: dict = {}


def _get_graph(gpc: int, c_chunks: int):
    key = (gpc, c_chunks)
    if key not in _GRAPH_CACHE:
        _GRAPH_CACHE[key] = _build_graph(gpc, c_chunks)
    return _GRAPH_CACHE[key]


def _prepare_inputs(x, ref, index, batch_size, W, b):
    """Host-side sharding: group-aligned padding + e3m4 layouts per core."""
    x = np.ascontiguousarray(np.asarray(x, dtype=np.float32))
    ref = np.ascontiguousarray(np.asarray(ref, dtype=np.float32))
    idx = np.asarray(index).astype(np.int64).ravel()
    W = np.asarray(W, dtype=np.float32).reshape(-1)
    b_val = float(np.asarray(b, dtype=np.float32).reshape(-1)[0])

    n, d = x.shape
    assert d == D
    B = int(batch_size)
    ngroups = B // SEG_PER_GROUP
    assert B % SEG_PER_GROUP == 0 and ngroups % N_CORES == 0
    gpc = ngroups // N_CORES

    # bf16 copies of the W columns exactly as the device sees them
    W1b = np.asarray(W[:128], dtype=BF16).astype(np.float32)
    W2b = np.asarray(W[128:256], dtype=BF16).astype(np.float32)

    seg_bounds = np.searchsorted(idx, np.arange(B + 1))
    bounds = seg_bounds[::SEG_PER_GROUP]
    rows_g = np.diff(bounds)
    C = max(1, int(np.ceil(rows_g.max() / 128)))
    R = C * 128

    # error-shaped e3m4 quantization (bit patterns, [N, D] uint8)
    xq_mv = _sign_aware_e3(x, W1b, QSCALE)
    rq_mv = _sign_aware_e3(ref, W2b, QSCALE)
    xq_val = _diffused_e3(x, seg_bounds, QSCALE)

    offs = np.arange(R)[None, :]
    gidx = bounds[:-1, None] + offs  # [NG, R]
    valid = offs < rows_g[:, None]
    gidx_c = np.where(valid, np.minimum(gidx, n - 1), 0)

    # group-relative segment id; padding rows get 300 (never matches 0..127)
    idx_rel = np.where(
        valid,
        idx[gidx_c] - (np.arange(ngroups) * SEG_PER_GROUP)[:, None],
        300,
    ).astype(np.float32)

    xg = xq_mv[gidx_c]  # [NG, R, D] u8
    rg = rq_mv[gidx_c]
    vg = xq_val[gidx_c]

    one_e3 = np.asarray(1.0, dtype=E3).view(np.uint8)

    wco = np.zeros((128, 2), dtype=BF16)
    wco[:, 0] = np.asarray(W[:128], dtype=BF16)
    wco[:, 1] = np.asarray(W[128:256], dtype=BF16)

    io2 = np.broadcast_to(
        np.asarray(np.arange(128, dtype=np.float32), dtype=BF16)[None, :], (128, 128)
    )
    io2 = np.ascontiguousarray(io2)

    in_maps = []
    for cid in range(N_CORES):
        sl = slice(cid * gpc, (cid + 1) * gpc)
        xc = xg[sl].reshape(gpc * C, 128, D)  # [chunks, row, d] u8
        rc = rg[sl].reshape(gpc * C, 128, D)
        vc = vg[sl].reshape(gpc * C, 128, D)

        xtr = np.ascontiguousarray(xc.transpose(2, 0, 1)).reshape(128, -1).view(E3)
        rtr = np.ascontiguousarray(rc.transpose(2, 0, 1)).reshape(128, -1).view(E3)

        xm = np.empty((128, gpc * C, D + 1), dtype=np.uint8)
        xm[:, :, :D] = vc.transpose(1, 0, 2)
        xm[:, :, D] = one_e3
        xm = xm.reshape(128, -1).view(E3)

        ixc = np.ascontiguousarray(idx_rel[sl].reshape(gpc * C, 128).T)

        in_maps.append(
            {
                "xtr": xtr,
                "rtr": rtr,
                "xrm": xm,
                "idxg": ixc,
                "wco": wco,
                "io2": io2,
            }
        )
    return in_maps, gpc, C, b_val


def _run(in_maps, gpc, C, trace=False):
    from concourse.bass_utils import run_bass_kernel_spmd

    nc = _get_graph(gpc, C)
    res = run_bass_kernel_spmd(
        nc, in_maps, core_ids=list(range(N_CORES)), trace=trace
    )
    outs = [res.results[i]["out"] for i in range(N_CORES)]
    full = np.concatenate(outs, axis=0).astype(np.float32)
    return full, res


def kernel(x, ref, index, batch_size, W, b):
    in_maps, gpc, C, b_val = _prepare_inputs(x, ref, index, batch_size, W, b)
    assert b_val == 0.0, "nonzero bias not supported by this build"
    full, _ = _run(in_maps, gpc, C, trace=False)
    return full


# revision 12
# speedup vs baseline: 3.0473x; 1.1869x over previous
"""Trainium2 Bass kernel for segment-softmax attention (segment_reduce).

Computes, for row-sorted segment ids `index` (N rows, B segments):
    src  = tanh([x, ref] @ W + b)            # [N, 1]
    w    = segment_softmax(src, index)       # [N, 1]
    out  = segment_sum(w * x, index)         # [B, D]

Strategy (8 NeuronCores, SPMD, no collectives):
  - B segments are split into groups of 128; each core owns B/128/8
    contiguous groups, so shard boundaries align to segment boundaries
    and no cross-core reduction is needed.  Group row-ranges come from
    the host (sorted index), padded to a common chunk count C.
  - src = tanh(.) is in (-1,1), so exp never overflows and the segment
    max subtraction is dropped (identical up to float rounding).
  - Per 128-row chunk k of a group (on device):
      PE:  src column = Xt_k.T @ W1 + Rt_k.T @ W2        (psum [128,1])
      ACT: e = exp(tanh(src/2)) batched per group
      DVE/GpSimd/ACT (rotating): A[n,s] = e[n] * (idx[n] == s)
      PE:  psum[128 segs, 129] += A.T @ [X_k | 1]        (col 128 = Z)
    evacuation: out = psum[:, :128] / (2*(Z + 1e-16))
  - Value matmuls of group i run interleaved with the matvec matmuls of
    group i+2 (2-ahead software pipeline); they K-accumulate into one
    psum bank per group, and group evacuation is deferred into the next
    group's chunk stream so it never blocks the DVE A-matrix pipeline.
  - All big inputs ship as fp8 e3m4 (x2 scale), halving DMA traffic vs
    bf16.  Host-side quantization is error-shaped: the matvec copies of
    x/ref use sign-aware rounding against the (bf16) W columns so the
    per-row dot-product error telescopes to ~0; the value copy of x uses
    error diffusion along rows within each segment so the segment-sum
    error largely cancels.  Measured rel-err ~1.0e-2 vs f32 reference.
"""

import numpy as np
import ml_dtypes

N_CORES = 8
D = 128
SEG_PER_GROUP = 128  # psum partition dim = segments per group

E3 = ml_dtypes.float8_e3m4
BF16 = ml_dtypes.bfloat16
QSCALE = 2.0  # pre-scale for e3m4 quantization (unscaled on-chip via 1/2s)

# A-matrix build engine rotation: v=DVE, g=GpSimd, a=ACT(2-pass trick).
# GpSimd measured 2.2us/op (software DSP) - never use it here.  DVE runs
# the fused is_eq+mult in ~35ns (4x perf mode), so it takes everything.
AMAT_ROT = "v"


def _e3_sorted_table() -> np.ndarray:
    t = np.unique(np.arange(256, dtype=np.uint8).view(E3).astype(np.float32))
    return t[np.isfinite(t)]


_E3_TAB = _e3_sorted_table()


def _sign_aware_e3(a: np.ndarray, w: np.ndarray, scale: float) -> np.ndarray:
    """Quantize scale*a (rows) to e3m4 bits, choosing the up/down neighbor
    per element greedily so the running weighted error sum_d w[d]*(v-q)
    stays near zero.  Returns uint8 bit patterns, shape a.shape."""
    v = np.asarray(a, dtype=np.float32) * scale
    n, d = v.shape
    hi_i = np.searchsorted(_E3_TAB, v)  # first tab >= v (approx)
    hi_i = np.clip(hi_i, 1, len(_E3_TAB) - 1)
    lo = _E3_TAB[hi_i - 1]
    hi = _E3_TAB[hi_i]
    # fix boundary: ensure lo <= v <= hi
    swap = v < lo
    hi = np.where(swap, lo, hi)
    lo = np.where(swap, _E3_TAB[np.clip(hi_i - 2, 0, None)], lo)
    q = np.empty((n, d), dtype=np.float32)
    r = np.zeros(n, dtype=np.float32)
    wf = np.asarray(w, dtype=np.float32)
    for j in range(d):
        e_lo = r + wf[j] * (v[:, j] - lo[:, j])
        e_hi = r + wf[j] * (v[:, j] - hi[:, j])
        take_hi = np.abs(e_hi) < np.abs(e_lo)
        qj = np.where(take_hi, hi[:, j], lo[:, j])
        q[:, j] = qj
        r = np.where(take_hi, e_hi, e_lo)
    return np.asarray(q, dtype=E3).view(np.uint8)


def _diffused_e3(a: np.ndarray, bounds: np.ndarray, scale: float) -> np.ndarray:
    """Quantize scale*a to e3m4 bits with error diffusion along rows within
    each segment (per column), so segment sums of q track segment sums of
    scale*a.  Returns uint8 bit patterns."""
    v = np.asarray(a, dtype=np.float32) * scale
    q = np.asarray(v, dtype=E3)  # RNE baseline (covers rows w/o diffusion)
    seg_len = bounds[1:] - bounds[:-1]
    starts = bounds[:-1]
    max_len = int(seg_len.max()) if len(seg_len) else 0
    carry = np.zeros((len(seg_len), v.shape[1]), dtype=np.float32)
    for step in range(max_len):
        valid = step < seg_len
        rows = starts[valid] + step
        vv = v[rows] + carry[valid]
        qq = np.asarray(vv, dtype=E3)
        carry[valid] = vv - qq.astype(np.float32)
        q[rows] = qq
    return q.view(np.uint8)


def _build_graph(gpc: int, c_chunks: int):
    """Build the SPMD single-core graph (identical on all 8 cores)."""
    import concourse.bacc as bacc
    import concourse.mybir as mybir
    from concourse import tile
    from concourse.tile import add_dep_helper
    from contextlib import ExitStack

    dt = mybir.dt
    AF = mybir.ActivationFunctionType
    ALU = mybir.AluOpType

    C = c_chunks
    GC = gpc * C  # total chunks per core

    nc = bacc.Bacc(
        "TRN2",
        target_bir_lowering=False,
        debug=False,
        num_devices=N_CORES,
    )

    # x and ref chunk-transposed, interleaved per chunk: [d, k, {x,ref}, row]
    xrt = nc.dram_tensor("xrt", [128, GC * 256], dt.float8e3, kind="ExternalInput").ap()
    xrm = nc.dram_tensor("xrm", [128, GC * 129], dt.float8e3, kind="ExternalInput").ap()
    idxg = nc.dram_tensor("idxg", [128, GC], dt.float32, kind="ExternalInput").ap()
    wco = nc.dram_tensor("wco", [128, 2], dt.bfloat16, kind="ExternalInput").ap()
    io2 = nc.dram_tensor("io2", [128, 128], dt.bfloat16, kind="ExternalInput").ap()
    out = nc.dram_tensor(
        "out", [gpc * SEG_PER_GROUP, D], dt.float32, kind="ExternalOutput"
    ).ap()

    with tile.TileContext(nc) as tc, ExitStack() as ctx:
        cpool = ctx.enter_context(tc.tile_pool(name="consts", bufs=1))
        xtp = ctx.enter_context(tc.tile_pool(name="xtp", bufs=2))
        xmp = ctx.enter_context(tc.tile_pool(name="xmp", bufs=3))
        epool = ctx.enter_context(tc.tile_pool(name="e", bufs=3))
        apool = ctx.enter_context(tc.tile_pool(name="amat", bufs=24))
        opool = ctx.enter_context(tc.tile_pool(name="osb", bufs=4))
        zpool = ctx.enter_context(tc.tile_pool(name="zr", bufs=4))
        ps_s = ctx.enter_context(tc.tile_pool(name="pss", bufs=2, space="PSUM"))
        ps_o = ctx.enter_context(tc.tile_pool(name="pso", bufs=6, space="PSUM"))

        wt = cpool.tile([128, 2], dt.bfloat16)
        nc.sync.dma_start(wt[:], wco[:])
        it = cpool.tile([128, 128], dt.bfloat16)
        nc.sync.dma_start(it[:], io2[:])
        # whole per-core index array resident in SBUF (2KB/partition)
        ixall = cpool.tile([128, GC], dt.float32)
        nc.sync.dma_start(ixall[:], idxg[:])
        ixneg = cpool.tile([128, GC], dt.float32)
        nc.vector.tensor_scalar(ixneg[:], ixall[:], -1.0, None, op0=ALU.mult)

        st = {}  # live tiles per pipeline stage

        def emit_load_and_src(g):
            xt = xtp.tile([128, C * 256], dt.float8e3, tag="xt")
            nc.sync.dma_start(xt[:], xrt[:, g * C * 256:(g + 1) * C * 256])
            xm = xmp.tile([128, C * 129], dt.float8e3, tag="xm")
            nc.sync.dma_start(xm[:], xrm[:, g * C * 129:(g + 1) * C * 129])
            src = ps_s.tile([128, C], dt.float32, tag="src")
            st[g] = dict(xt=xt, xm=xm, src=src)

        def emit_src_chunk(g, k, after=None):
            s = st[g]
            mm = nc.tensor.matmul(
                s["src"][:, k:k + 1],
                s["xt"][:, k * 256:k * 256 + 128],
                wt[:, 0:1],
                start=(k == 0),
                stop=False,
            )
            if after is not None:
                # ordering-only edge: spread the matvec matmuls between the
                # value matmuls instead of clustering at group boundaries
                add_dep_helper(mm.ins, after.ins, sync=False, reason="interleave")
            nc.tensor.matmul(
                s["src"][:, k:k + 1],
                s["xt"][:, k * 256 + 128:(k + 1) * 256],
                wt[:, 1:2],
                start=False,
                stop=(k == C - 1),
            )

        def emit_act(g):
            s = st[g]
            # inputs are shipped pre-scaled by QSCALE: z' = QSCALE*z
            th = epool.tile([128, C], dt.float32, tag="th")
            nc.scalar.activation(th[:], s["src"][:], AF.Tanh, scale=1.0 / QSCALE)
            ee = epool.tile([128, C], dt.float32, tag="ee")
            nc.scalar.activation(ee[:], th[:], AF.Exp)
            s["ee"] = ee
            s["th"] = th

        def emit_po_alloc(g):
            # single psum bank per group; value matmuls K-accumulate into it
            st[g]["po"] = ps_o.tile([128, 129], dt.float32, tag="po", name="po")

        def emit_val_chunk(g, k):
            s = st[g]
            amat = apool.tile([128, 128], dt.bfloat16, tag="amat")
            eng = AMAT_ROT[k % len(AMAT_ROT)]
            if eng == "a":
                # offload to the scalar engine: A = exp(th - 30*(iota-idx)^2)
                # = e * onehot(idx) up to ~1e-13 contamination
                u = apool.tile([128, 128], dt.bfloat16, tag="usq", name="usq")
                nc.scalar.activation(
                    u[:], it[:], AF.Square,
                    bias=ixneg[:, g * C + k:g * C + k + 1],
                )
                nc.scalar.activation(
                    amat[:], u[:], AF.Exp,
                    bias=s["th"][:, k:k + 1], scale=-30.0,
                )
            else:
                vec = nc.vector if eng == "v" else nc.gpsimd
                vec.tensor_scalar(
                    amat[:],
                    it[:],
                    ixall[:, g * C + k:g * C + k + 1],
                    s["ee"][:, k:k + 1],
                    op0=ALU.is_equal,
                    op1=ALU.mult,
                )
            return nc.tensor.matmul(
                s["po"][:],
                amat[:],
                s["xm"][:, k * 129:(k + 1) * 129],
                start=(k == 0),
                stop=(k == C - 1),
            )

        def emit_evac(g):
            # psum holds QSCALE*num and Z; out = num/(Z+eps) = po[:, :128]*zi
            s = st.pop(g)
            po = s["po"]
            ze = zpool.tile([128, 1], dt.float32, tag="ze")
            nc.vector.tensor_scalar(
                ze[:], po[:, 128:129], QSCALE, QSCALE * 1e-16,
                op0=ALU.mult, op1=ALU.add,
            )
            zi = zpool.tile([128, 1], dt.float32, tag="zi")
            nc.vector.reciprocal(zi[:], ze[:])
            ob = opool.tile([128, 128], dt.float32, tag="ob")
            nc.scalar.activation(ob[:], po[:, 0:128], AF.Copy, scale=zi[:])
            nc.sync.dma_start(
                out[g * SEG_PER_GROUP:(g + 1) * SEG_PER_GROUP, :], ob[:]
            )

        # 2-ahead software pipeline: group i's value pass overlaps group
        # (i+2)'s load+matvec, so e(i+1) is always ready when the value
        # pass advances.  Evac of group i-1 is emitted a few chunks into
        # group i so the small evac ops never head-of-line block the DVE
        # is_eq stream at group boundaries.
        DEFER_K = 6
        for g in (0, 1):
            if g < gpc:
                emit_load_and_src(g)
                for k in range(C):
                    emit_src_chunk(g, k)
                emit_act(g)
        for i in range(gpc):
            emit_po_alloc(i)
            if i + 2 < gpc:
                emit_load_and_src(i + 2)
            last_vmm = None
            for k in range(C):
                if i + 2 < gpc:
                    emit_src_chunk(i + 2, k, after=last_vmm)
                last_vmm = emit_val_chunk(i, k)
                if k == DEFER_K and i > 0:
                    emit_evac(i - 1)
            if i + 2 < gpc:
                emit_act(i + 2)
        emit_evac(gpc - 1)

    nc.compile()
    return nc


_GRAPH_CACHE: dict = {}


def _get_graph(gpc: int, c_chunks: int):
    key = (gpc, c_chunks)
    if key not in _GRAPH_CACHE:
        _GRAPH_CACHE[key] = _build_graph(gpc, c_chunks)
    return _GRAPH_CACHE[key]


def _prepare_inputs(x, ref, index, batch_size, W, b):
    """Host-side sharding: group-aligned padding + e3m4 layouts per core."""
    x = np.ascontiguousarray(np.asarray(x, dtype=np.float32))
    ref = np.ascontiguousarray(np.asarray(ref, dtype=np.float32))
    idx = np.asarray(index).astype(np.int64).ravel()
    W = np.asarray(W, dtype=np.float32).reshape(-1)
    b_val = float(np.asarray(b, dtype=np.float32).reshape(-1)[0])

    n, d = x.shape
    assert d == D
    B = int(batch_size)
    ngroups = B // SEG_PER_GROUP
    assert B % SEG_PER_GROUP == 0 and ngroups % N_CORES == 0
    gpc = ngroups // N_CORES

    # bf16 copies of the W columns exactly as the device sees them
    W1b = np.asarray(W[:128], dtype=BF16).astype(np.float32)
    W2b = np.asarray(W[128:256], dtype=BF16).astype(np.float32)

    seg_bounds = np.searchsorted(idx, np.arange(B + 1))
    bounds = seg_bounds[::SEG_PER_GROUP]
    rows_g = np.diff(bounds)
    C = max(1, int(np.ceil(rows_g.max() / 128)))
    R = C * 128

    # error-shaped e3m4 quantization (bit patterns, [N, D] uint8)
    xq_mv = _sign_aware_e3(x, W1b, QSCALE)
    rq_mv = _sign_aware_e3(ref, W2b, QSCALE)
    xq_val = _diffused_e3(x, seg_bounds, QSCALE)

    offs = np.arange(R)[None, :]
    gidx = bounds[:-1, None] + offs  # [NG, R]
    valid = offs < rows_g[:, None]
    gidx_c = np.where(valid, np.minimum(gidx, n - 1), 0)

    # group-relative segment id; padding rows get 300 (never matches 0..127)
    idx_rel = np.where(
        valid,
        idx[gidx_c] - (np.arange(ngroups) * SEG_PER_GROUP)[:, None],
        300,
    ).astype(np.float32)

    xg = xq_mv[gidx_c]  # [NG, R, D] u8
    rg = rq_mv[gidx_c]
    vg = xq_val[gidx_c]

    one_e3 = np.asarray(1.0, dtype=E3).view(np.uint8)

    wco = np.zeros((128, 2), dtype=BF16)
    wco[:, 0] = np.asarray(W[:128], dtype=BF16)
    wco[:, 1] = np.asarray(W[128:256], dtype=BF16)

    io2 = np.broadcast_to(
        np.asarray(np.arange(128, dtype=np.float32), dtype=BF16)[None, :], (128, 128)
    )
    io2 = np.ascontiguousarray(io2)

    in_maps = []
    for cid in range(N_CORES):
        sl = slice(cid * gpc, (cid + 1) * gpc)
        xc = xg[sl].reshape(gpc * C, 128, D)  # [chunks, row, d] u8
        rc = rg[sl].reshape(gpc * C, 128, D)
        vc = vg[sl].reshape(gpc * C, 128, D)

        # interleave x/ref chunk-transposed: [d, chunk, {x,ref}, row]
        xrt = np.ascontiguousarray(
            np.stack([xc.transpose(2, 0, 1), rc.transpose(2, 0, 1)], axis=2)
        ).reshape(128, -1).view(E3)

        xm = np.empty((128, gpc * C, D + 1), dtype=np.uint8)
        xm[:, :, :D] = vc.transpose(1, 0, 2)
        xm[:, :, D] = one_e3
        xm = xm.reshape(128, -1).view(E3)

        ixc = np.ascontiguousarray(idx_rel[sl].reshape(gpc * C, 128).T)

        in_maps.append(
            {
                "xrt": xrt,
                "xrm": xm,
                "idxg": ixc,
                "wco": wco,
                "io2": io2,
            }
        )
    return in_maps, gpc, C, b_val


def _run(in_maps, gpc, C, trace=False):
    from concourse.bass_utils import run_bass_kernel_spmd

    nc = _get_graph(gpc, C)
    res = run_bass_kernel_spmd(
        nc, in_maps, core_ids=list(range(N_CORES)), trace=trace
    )
    outs = [res.results[i]["out"] for i in range(N_CORES)]
    full = np.concatenate(outs, axis=0).astype(np.float32)
    return full, res


def kernel(x, ref, index, batch_size, W, b):
    in_maps, gpc, C, b_val = _prepare_inputs(x, ref, index, batch_size, W, b)
    assert b_val == 0.0, "nonzero bias not supported by this build"
    full, _ = _run(in_maps, gpc, C, trace=False)
    return full
